# revision 6
# baseline (speedup 1.0000x reference)
import sys

import numpy as np

try:
    import concourse.bass as bass
except ImportError:
    sys.path.insert(0, "/opt/trn_rl_repo")
    import concourse.bass as bass

import ml_dtypes

import concourse.bacc as bacc
import concourse.mybir as mybir
import concourse.tile as tile
from concourse.bass_utils import run_bass_kernel_spmd

F32 = mybir.dt.float32
BF16 = mybir.dt.bfloat16
NP_BF16 = np.dtype(ml_dtypes.bfloat16)
B, S, D = 2, 2048, 1024
NH, DH = 16, 64
HPC = 4            # heads per core
HF = HPC * DH      # 256 per-core head features
TQ = S // 4        # 512: t-chunk / i-chunk quarter
NJT = S // 128     # 16 j-tiles of 128
SCALE = 1.0 / float(np.sqrt(DH))

_CACHE = {}


def _build_graph(variant="full", reps=1):
    nc = bacc.Bacc(num_devices=8)

    xqT = nc.dram_tensor("xqT", [D, S], BF16, kind="ExternalInput")
    xkT = nc.dram_tensor("xkT", [D, S], BF16, kind="ExternalInput")
    xvT = nc.dram_tensor("xvT", [D, S], BF16, kind="ExternalInput")
    wqT = nc.dram_tensor("wqT", [D, HF], BF16, kind="ExternalInput")
    wkT = nc.dram_tensor("wkT", [D, HF], BF16, kind="ExternalInput")
    wvT = nc.dram_tensor("wvT", [D, HF], BF16, kind="ExternalInput")
    # Wo.T column slice for this core's 256 output features
    woT = nc.dram_tensor("woT", [D, HF], BF16, kind="ExternalInput")
    dmask = nc.dram_tensor("dmask", [128, 128], F32, kind="ExternalInput")
    # full rows, this core's 256-feature slice of the output
    out_q = nc.dram_tensor("out_q", [S, HF], F32, kind="ExternalOutput")

    Exp = mybir.ActivationFunctionType.Exp

    with tile.TileContext(nc) as tc:
        with (
            tc.tile_pool(name="dram", bufs=1, space="DRAM") as dramp,
            tc.tile_pool(name="const", bufs=1) as constp,
            tc.tile_pool(name="persist", bufs=1) as pers,
            tc.tile_pool(name="weights", bufs=1) as wpool,
            tc.tile_pool(name="xstage", bufs=6) as xpool,
            tc.tile_pool(name="attn", bufs=4) as apool,
            tc.tile_pool(name="ctx", bufs=2) as cpool,
            tc.tile_pool(name="cstage", bufs=2) as cstp,
            tc.tile_pool(name="rb", bufs=2) as rbpool,
            tc.tile_pool(name="rv", bufs=2) as rvpool,
            tc.tile_pool(name="obuf", bufs=3) as obp,
            tc.tile_pool(name="ps_mm", bufs=2, space="PSUM") as ps_mm,
            tc.tile_pool(name="ps_s", bufs=3, space="PSUM") as ps_s,
            tc.tile_pool(name="ps_ctx", bufs=2, space="PSUM") as ps_ctx,
            tc.tile_pool(name="ps_b", bufs=1, space="PSUM") as ps_b,
        ):
            ccin = [dramp.tile([HF, TQ], BF16, name=f"ccin{j}") for j in range(4)]
            agout = [
                dramp.tile([4 * HF, TQ], BF16, name=f"agout{j}") for j in range(4)
            ]

            dmask_sb = constp.tile([128, 128], F32, name="dmask_sb")
            nc.sync.dma_start(dmask_sb[:], dmask[:, :])
            ones_sb = constp.tile([1, DH], BF16, name="ones_sb")
            nc.vector.memset(ones_sb[:], 1.0)

            wq_sb = wpool.tile([128, 8, HF], BF16, name="wq_sb")
            wk_sb = wpool.tile([128, 8, HF], BF16, name="wk_sb")
            wv_sb = wpool.tile([128, 8, HF], BF16, name="wv_sb")
            nc.sync.dma_start(wq_sb[:], wqT[:, :].rearrange("(n p) o -> p n o", p=128))
            nc.sync.dma_start(wk_sb[:], wkT[:, :].rearrange("(n p) o -> p n o", p=128))
            nc.sync.dma_start(wv_sb[:], wvT[:, :].rearrange("(n p) o -> p n o", p=128))
            wo_sb = wpool.tile([128, 8, HF], BF16, name="wo_sb")
            nc.sync.dma_start(wo_sb[:], woT[:, :].rearrange("(n p) d -> p n d", p=128))

            # Persistent Q^T/K^T (2 tiles each: heads (0,1) and (2,3) stacked on
            # partitions) and V in natural orientation augmented with a ones
            # column (row 64 of the AV product becomes the softmax denominator).
            QT = [pers.tile([128, S], BF16, name=f"QT{u}") for u in range(2)]
            KT = [pers.tile([128, S], BF16, name=f"KT{u}") for u in range(2)]
            Vb = pers.tile([128, NJT * HPC, DH + 1], BF16, name="Vb")
            nc.vector.memset(Vb[:, :, DH], 1.0)

            def oproj(ic, cst):
                # out rows of chunk ic x this core's 256 features
                for tt in range(4):
                    pso = ps_mm.tile([128, TQ], F32, name="ps")
                    for kt in range(8):
                        nc.tensor.matmul(
                            pso[:, 0:HF],
                            cst[:, kt, bass.ts(tt, 128)],
                            wo_sb[:, kt, :],
                            start=(kt == 0),
                            stop=(kt == 7),
                        )
                    ob = obp.tile([128, HF], F32, name="ob")
                    nc.scalar.copy(ob[:], pso[:, 0:HF])
                    nc.sync.dma_start(
                        out_q[ic * TQ + tt * 128:ic * TQ + (tt + 1) * 128, :],
                        ob[:],
                    )

            for rep in range(reps):
             csts = {}
             for tcc in range(4):
                xq_sb = xpool.tile([128, 8, TQ], BF16, name="xst")
                xk_sb = xpool.tile([128, 8, TQ], BF16, name="xst")
                xv_sb = xpool.tile([128, 8, TQ], BF16, name="xst")
                for xs, src in ((xq_sb, xqT), (xk_sb, xkT), (xv_sb, xvT)):
                    src_ap = src[:, bass.ts(tcc, TQ)].rearrange(
                        "(n p) t -> p n t", p=128
                    )
                    nc.sync.dma_start(xs[:], src_ap)

                # Q/K projections for this t-chunk: psum[o128, t512] over 8 d-tiles
                for xs, w_sb, dst in ((xq_sb, wq_sb, QT), (xk_sb, wk_sb, KT)):
                    for u in range(2):
                        ps = ps_mm.tile([128, TQ], F32, name="ps")
                        for kd in range(8):
                            nc.tensor.matmul(
                                ps[:],
                                w_sb[:, kd, bass.ts(u, 128)],
                                xs[:, kd, :],
                                start=(kd == 0),
                                stop=(kd == 7),
                            )
                        nc.vector.tensor_copy(dst[u][:, bass.ts(tcc, TQ)], ps[:])

                # V projection: natural orientation [t128, feat256] per j-tile
                for jl in range(4):
                    jt = tcc * 4 + jl
                    psv = ps_mm.tile([128, TQ], F32, name="ps")
                    for kd in range(8):
                        nc.tensor.matmul(
                            psv[:, 0:HF],
                            xv_sb[:, kd, bass.ts(jl, 128)],
                            wv_sb[:, kd, :],
                            start=(kd == 0),
                            stop=(kd == 7),
                        )
                    nc.vector.tensor_copy(
                        Vb[:, jt * HPC:(jt + 1) * HPC, 0:DH],
                        psv[:, 0:HF].rearrange("p (h k) -> p h k", k=DH),
                    )

                # Attention for i-chunk ic == tcc (all K/V up to j<=i now exist)
                ic = tcc
                n_jt = 4 * ic + 4
                skew = 1
                for h in range(HPC):
                    u, po = h // 2, (h % 2) * DH
                    pctx = ps_ctx.tile([DH + 1, TQ], F32, name="pctx")
                    ats = []
                    los = []
                    for jt in range(n_jt):
                        p = jt - 4 * ic
                        lo = max(p, 0) * 128
                        los.append(lo)
                        ps = ps_s.tile([128, TQ], F32, name="ps_sc")
                        nc.tensor.matmul(
                            ps[:, lo:TQ],
                            KT[u][po:po + DH, bass.ts(jt, 128)],
                            QT[u][po:po + DH, ic * TQ + lo:(ic + 1) * TQ],
                            start=True,
                            stop=True,
                        )
                        at = apool.tile([128, TQ], BF16, name="at")
                        if p >= 0:
                            nc.vector.tensor_add(
                                ps[:, bass.ts(p, 128)],
                                ps[:, bass.ts(p, 128)],
                                dmask_sb[:],
                            )
                        nc.scalar.activation(
                            at[:, lo:TQ], ps[:, lo:TQ], Exp, scale=SCALE
                        )
                        ats.append(at)
                        # AV accumulation skewed behind scores for PE/ACT
                        # pipelining
                        if jt >= skew:
                            pv = jt - skew
                            nc.tensor.matmul(
                                pctx[:, los[pv]:TQ],
                                Vb[:, pv * HPC + h, :],
                                ats[pv][:, los[pv]:TQ],
                                start=(pv == 0),
                                stop=False,
                            )
                    for pv in range(max(n_jt - skew, 0), n_jt):
                        nc.tensor.matmul(
                            pctx[:, los[pv]:TQ],
                            Vb[:, pv * HPC + h, :],
                            ats[pv][:, los[pv]:TQ],
                            start=(pv == 0),
                            stop=(pv == n_jt - 1),
                        )

                    # Normalize: row DH of pctx is the denominator. Broadcast
                    # 1/denom across 64 partitions via a rank-1 matmul.
                    rv = rvpool.tile([1, TQ], BF16, name="rvec")
                    with nc.allow_low_precision(reason="bf16 denom ok at 2e-2"):
                        nc.vector.reciprocal(rv[:], pctx[DH:DH + 1, :])
                    pb = ps_b.tile([DH, TQ], F32, name="pb")
                    nc.tensor.matmul(pb[:], ones_sb[:], rv[:], start=True, stop=True)
                    rb = rbpool.tile([DH, TQ], F32, name="rbt")
                    nc.vector.tensor_copy(rb[:], pb[:])
                    ctxT = cpool.tile([DH, TQ], BF16, name="ctxT")
                    nc.vector.tensor_mul(ctxT[:], pctx[0:DH, :], rb[:])
                    nc.sync.dma_start(
                        ccin[ic][h * DH:(h + 1) * DH, :], ctxT[:]
                    )

                # Gather all 16 heads' ctxT for this i-chunk across the 4-core
                # group (concat by group rank = head-major feature order).
                nc.gpsimd.collective_compute(
                    "AllGather",
                    mybir.AluOpType.bypass,
                    replica_groups=[[0, 1, 2, 3], [4, 5, 6, 7]],
                    ins=[ccin[ic].opt()],
                    outs=[agout[ic].opt()],
                )
                # Stage the gathered full-feature ctx for chunk ic (DMA queue,
                # overlaps with next chunk's compute)...
                cst = cstp.tile([128, 8, TQ], BF16, name="cst")
                nc.sync.dma_start(
                    cst[:], agout[ic][:, :].rearrange("(n p) t -> p n t", p=128)
                )
                csts[ic] = cst
                # ...and emit the PREVIOUS chunk's output projection so the PE
                # never waits on this chunk's AllGather.
                if ic >= 1:
                    oproj(ic - 1, csts.pop(ic - 1))
             oproj(3, csts.pop(3))

    nc.finalize()
    return nc


def _make_in_maps(inputs):
    query, key, value = inputs["query"], inputs["key"], inputs["value"]
    mask = inputs["mask"]
    Wq, Wk, Wv, Wo = inputs["Wq"], inputs["Wk"], inputs["Wv"], inputs["Wo"]

    dmask_blk = np.where(
        np.asarray(mask[:128, :128]).T, np.float32(0.0), np.float32(-1e9)
    ).astype(np.float32)
    woT_full = np.ascontiguousarray(np.asarray(Wo, np.float32).T.astype(NP_BF16))

    def bt(a):
        return np.ascontiguousarray(np.asarray(a, np.float32).T.astype(NP_BF16))

    in_maps = []
    for c in range(8):
        b, r = divmod(c, 4)
        rs = slice(r * HF, (r + 1) * HF)
        in_maps.append(
            {
                "xqT": bt(query[b]),
                "xkT": bt(key[b]),
                "xvT": bt(value[b]),
                "wqT": bt(Wq[rs]),
                "wkT": bt(Wk[rs]),
                "wvT": bt(Wv[rs]),
                "woT": np.ascontiguousarray(woT_full[:, rs]),
                "dmask": dmask_blk,
            }
        )
    return in_maps


def _gather_out(full):
    """full: [8*S, HF] concat of per-core out_q along axis 0."""
    out = np.empty((B, S, D), np.float32)
    for c in range(8):
        b, r = divmod(c, 4)
        out[b, :, r * HF:(r + 1) * HF] = full[c * S:(c + 1) * S]
    return out


def _run(inputs, trace=False):
    if "nc" not in _CACHE:
        _CACHE["nc"] = _build_graph()
    nc = _CACHE["nc"]
    in_maps = _make_in_maps(inputs)
    res = run_bass_kernel_spmd(nc, in_maps, core_ids=list(range(8)), trace=trace)

    out = np.empty((B, S, D), np.float32)
    for c in range(8):
        b, r = divmod(c, 4)
        out[b, :, r * HF:(r + 1) * HF] = np.asarray(res.results[c]["out_q"])
    return out, res


def kernel(**inputs):
    out, _ = _run(inputs, trace=False)
    return out


# revision 15
# speedup vs baseline: 1.8236x; 1.8236x over previous
import sys

import numpy as np

try:
    import concourse.bass as bass
except ImportError:
    sys.path.insert(0, "/opt/trn_rl_repo")
    import concourse.bass as bass

import ml_dtypes

import concourse.bacc as bacc
import concourse.mybir as mybir
import concourse.tile as tile
from concourse.bass_utils import run_bass_kernel_spmd

F32 = mybir.dt.float32
BF16 = mybir.dt.bfloat16
NP_BF16 = np.dtype(ml_dtypes.bfloat16)
B, S, D = 2, 2048, 1024
NH, DH = 16, 64
HPC = 4            # heads per core
HF = HPC * DH      # 256 per-core head features
TQ = S // 4        # 512: t-chunk / i-chunk quarter
NJT = S // 128     # 16 j-tiles of 128
SCALE = 1.0 / float(np.sqrt(DH))

_CACHE = {}


def _build_graph(variant="full", reps=1):
    nc = bacc.Bacc(num_devices=8)

    xqT = nc.dram_tensor("xqT", [D, S], BF16, kind="ExternalInput")
    xkT = nc.dram_tensor("xkT", [D, S], BF16, kind="ExternalInput")
    xvT = nc.dram_tensor("xvT", [D, S], BF16, kind="ExternalInput")
    wqT = nc.dram_tensor("wqT", [D, HF], BF16, kind="ExternalInput")
    wkT = nc.dram_tensor("wkT", [D, HF], BF16, kind="ExternalInput")
    wvT = nc.dram_tensor("wvT", [D, HF], BF16, kind="ExternalInput")
    # Wo.T rows for this core's 256 ctx features (row-sharded Wo)
    woT = nc.dram_tensor("woT", [HF, D], BF16, kind="ExternalInput")
    dmask = nc.dram_tensor("dmask", [128, 128], F32, kind="ExternalInput")
    # chunk-major [ic, f, t]: this core's 256-feature ReduceScatter shard
    out_q = nc.dram_tensor("out_q", [4, HF, TQ], BF16, kind="ExternalOutput")

    Exp = mybir.ActivationFunctionType.Exp

    with tile.TileContext(nc) as tc:
        with (
            tc.tile_pool(name="dram", bufs=1, space="DRAM") as dramp,
            tc.tile_pool(name="const", bufs=1) as constp,
            tc.tile_pool(name="persist", bufs=1) as pers,
            tc.tile_pool(name="weights", bufs=1) as wpool,
            tc.tile_pool(name="xstage", bufs=6) as xpool,
            tc.tile_pool(name="attn", bufs=5) as apool,
            tc.tile_pool(name="ctx", bufs=4) as cpool,
            tc.tile_pool(name="rb", bufs=2) as rbpool,
            tc.tile_pool(name="rv", bufs=2) as rvpool,
            tc.tile_pool(name="obuf", bufs=3) as obp,
            tc.tile_pool(name="ps_mm", bufs=2, space="PSUM") as ps_mm,
            tc.tile_pool(name="ps_s", bufs=3, space="PSUM") as ps_s,
            tc.tile_pool(name="ps_ctx", bufs=2, space="PSUM") as ps_ctx,
            tc.tile_pool(name="ps_b", bufs=1, space="PSUM") as ps_b,
        ):
            # per-chunk partial out-proj [f, t] (bf16) to be ReduceScattered
            po_c = [dramp.tile([D, TQ], BF16, name=f"po{j}") for j in range(4)]
            # RS can't write IO tensors directly; land in Shared scratch then
            # DMA to out_q.
            ro_c = [dramp.tile([HF, TQ], BF16, name=f"ro{j}") for j in range(4)]

            # Weights/mask staged from the ACT queue so the startup x loads
            # don't serialize behind them on SP.
            dmask_sb = constp.tile([128, 128], F32, name="dmask_sb")
            nc.scalar.dma_start(dmask_sb[:], dmask[:, :])
            ones_sb = constp.tile([1, DH], BF16, name="ones_sb")
            nc.vector.memset(ones_sb[:], 1.0)

            wq_sb = wpool.tile([128, 8, HF], BF16, name="wq_sb")
            wk_sb = wpool.tile([128, 8, HF], BF16, name="wk_sb")
            wv_sb = wpool.tile([128, 8, HF], BF16, name="wv_sb")
            nc.scalar.dma_start(wq_sb[:], wqT[:, :].rearrange("(n p) o -> p n o", p=128))
            nc.scalar.dma_start(wk_sb[:], wkT[:, :].rearrange("(n p) o -> p n o", p=128))
            nc.scalar.dma_start(wv_sb[:], wvT[:, :].rearrange("(n p) o -> p n o", p=128))
            wo_sb = wpool.tile([128, 2, D], BF16, name="wo_sb")
            nc.scalar.dma_start(wo_sb[:], woT[:, :].rearrange("(u p) d -> p u d", p=128))

            # Persistent Q^T/K^T (2 tiles each: heads (0,1) and (2,3) stacked on
            # partitions) and V in natural orientation augmented with a ones
            # column (row 64 of the AV product becomes the softmax denominator).
            QT = [pers.tile([128, S], BF16, name=f"QT{u}") for u in range(2)]
            KT = [pers.tile([128, S], BF16, name=f"KT{u}") for u in range(2)]
            Vb = pers.tile([128, NJT * HPC, DH + 1], BF16, name="Vb")
            nc.vector.memset(Vb[:, :, DH], 1.0)

            for rep in range(reps):
             for tcc in range(4):
                xq_sb = xpool.tile([128, 8, TQ], BF16, name="xst")
                xk_sb = xpool.tile([128, 8, TQ], BF16, name="xst")
                xv_sb = xpool.tile([128, 8, TQ], BF16, name="xst")
                # Spread the three 1MB x loads across DMA-issue queues (they
                # serialize per queue): q on SP, k/v on the gpsimd queue.
                for xs, src, eng in (
                    (xq_sb, xqT, nc.sync),
                    (xk_sb, xkT, nc.gpsimd),
                    (xv_sb, xvT, nc.gpsimd),
                ):
                    src_ap = src[:, bass.ts(tcc, TQ)].rearrange(
                        "(n p) t -> p n t", p=128
                    )
                    eng.dma_start(xs[:], src_ap)

                # Q/K projections for this t-chunk: psum[o128, t512] over 8 d-tiles
                for xs, w_sb, dst in ((xq_sb, wq_sb, QT), (xk_sb, wk_sb, KT)):
                    for u in range(2):
                        ps = ps_mm.tile([128, TQ], F32, name="ps")
                        for kd in range(8):
                            nc.tensor.matmul(
                                ps[:],
                                w_sb[:, kd, bass.ts(u, 128)],
                                xs[:, kd, :],
                                start=(kd == 0),
                                stop=(kd == 7),
                            )
                        nc.vector.tensor_copy(dst[u][:, bass.ts(tcc, TQ)], ps[:])

                # V projection: natural orientation [t128, feat256] per j-tile
                for jl in range(4):
                    jt = tcc * 4 + jl
                    psv = ps_mm.tile([128, TQ], F32, name="ps")
                    for kd in range(8):
                        nc.tensor.matmul(
                            psv[:, 0:HF],
                            xv_sb[:, kd, bass.ts(jl, 128)],
                            wv_sb[:, kd, :],
                            start=(kd == 0),
                            stop=(kd == 7),
                        )
                    nc.vector.tensor_copy(
                        Vb[:, jt * HPC:(jt + 1) * HPC, 0:DH],
                        psv[:, 0:HF].rearrange("p (h k) -> p h k", k=DH),
                    )

                # Attention for i-chunk ic == tcc (all K/V up to j<=i now exist)
                ic = tcc
                n_jt = 4 * ic + 4
                skew = 2
                # normalized ctx for this chunk, head pairs stacked: ctx2[u]
                # partition p holds ctx feature u*128+p (head-major)
                ctx2 = [cpool.tile([128, TQ], BF16, name="ctx2") for _ in range(2)]

                def norm_tail(h, pctx, rv):
                    # Broadcast 1/denom across 64 partitions via a rank-1
                    # matmul, then scale this head's ctx into ctx2.
                    pb = ps_b.tile([DH, TQ], F32, name="pb")
                    nc.tensor.matmul(pb[:], ones_sb[:], rv[:], start=True, stop=True)
                    rb = rbpool.tile([DH, TQ], F32, name="rbt")
                    nc.vector.tensor_copy(rb[:], pb[:])
                    po = (h % 2) * DH
                    nc.vector.tensor_mul(
                        ctx2[h // 2][po:po + DH, :], pctx[0:DH, :], rb[:]
                    )

                pending = None
                for h in range(HPC):
                    u, po = h // 2, (h % 2) * DH
                    pctx = ps_ctx.tile([DH + 1, TQ], F32, name="pctx")
                    ats = []
                    los = []
                    for jt in range(n_jt):
                        p = jt - 4 * ic
                        lo = max(p, 0) * 128
                        los.append(lo)
                        ps = ps_s.tile([128, TQ], F32, name="ps_sc")
                        nc.tensor.matmul(
                            ps[:, lo:TQ],
                            KT[u][po:po + DH, bass.ts(jt, 128)],
                            QT[u][po:po + DH, ic * TQ + lo:(ic + 1) * TQ],
                            start=True,
                            stop=True,
                        )
                        at = apool.tile([128, TQ], BF16, name="at")
                        if p >= 0:
                            nc.vector.tensor_add(
                                ps[:, bass.ts(p, 128)],
                                ps[:, bass.ts(p, 128)],
                                dmask_sb[:],
                            )
                        nc.scalar.activation(
                            at[:, lo:TQ], ps[:, lo:TQ], Exp, scale=SCALE
                        )
                        ats.append(at)
                        if jt == 1 and pending is not None:
                            # Previous head's normalization: its reciprocal has
                            # had a whole head-gap to finish, so the rank-1
                            # matmul won't stall the PE here.
                            norm_tail(*pending)
                            pending = None
                        # AV accumulation skewed behind scores for PE/ACT
                        # pipelining
                        if jt >= skew:
                            pv = jt - skew
                            nc.tensor.matmul(
                                pctx[:, los[pv]:TQ],
                                Vb[:, pv * HPC + h, :],
                                ats[pv][:, los[pv]:TQ],
                                start=(pv == 0),
                                stop=False,
                            )
                    for pv in range(max(n_jt - skew, 0), n_jt):
                        nc.tensor.matmul(
                            pctx[:, los[pv]:TQ],
                            Vb[:, pv * HPC + h, :],
                            ats[pv][:, los[pv]:TQ],
                            start=(pv == 0),
                            stop=(pv == n_jt - 1),
                        )

                    # Row DH of pctx is the softmax denominator.
                    rv = rvpool.tile([1, TQ], BF16, name="rvec")
                    with nc.allow_low_precision(reason="bf16 denom ok at 2e-2"):
                        nc.vector.reciprocal(rv[:], pctx[DH:DH + 1, :])
                    pending = (h, pctx, rv)
                norm_tail(*pending)

                # Partial output projection for this chunk: contract this
                # core's 256 ctx features against its 256 rows of Wo.T,
                # producing all 1024 output features [f, t] in bf16.
                for ff in range(8):
                    pso = ps_mm.tile([128, TQ], F32, name="ps")
                    for u in range(2):
                        nc.tensor.matmul(
                            pso[:],
                            wo_sb[:, u, bass.ts(ff, 128)],
                            ctx2[u][:],
                            start=(u == 0),
                            stop=(u == 1),
                        )
                    ob = obp.tile([128, TQ], BF16, name="ob")
                    nc.vector.tensor_copy(ob[:], pso[:])
                    nc.sync.dma_start(po_c[ic][bass.ts(ff, 128), :], ob[:])

                # Sum partials across the 4-core group; rank r receives
                # feature rows [r*256, (r+1)*256) directly into out_q[ic].
                nc.gpsimd.collective_compute(
                    "ReduceScatter",
                    mybir.AluOpType.add,
                    replica_groups=[[0, 1, 2, 3], [4, 5, 6, 7]],
                    ins=[po_c[ic].opt()],
                    outs=[ro_c[ic].opt()],
                )
                nc.sync.dma_start(out_q[ic], ro_c[ic][:, :])

    nc.finalize()
    return nc


def _make_in_maps(inputs):
    query, key, value = inputs["query"], inputs["key"], inputs["value"]
    mask = inputs["mask"]
    Wq, Wk, Wv, Wo = inputs["Wq"], inputs["Wk"], inputs["Wv"], inputs["Wo"]

    dmask_blk = np.where(
        np.asarray(mask[:128, :128]).T, np.float32(0.0), np.float32(-1e9)
    ).astype(np.float32)
    woT_full = np.ascontiguousarray(np.asarray(Wo, np.float32).T.astype(NP_BF16))

    def bt(a):
        return np.ascontiguousarray(np.asarray(a, np.float32).T.astype(NP_BF16))

    in_maps = []
    for c in range(8):
        b, r = divmod(c, 4)
        rs = slice(r * HF, (r + 1) * HF)
        in_maps.append(
            {
                "xqT": bt(query[b]),
                "xkT": bt(key[b]),
                "xvT": bt(value[b]),
                "wqT": bt(Wq[rs]),
                "wkT": bt(Wk[rs]),
                "wvT": bt(Wv[rs]),
                "woT": np.ascontiguousarray(woT_full[rs, :]),
                "dmask": dmask_blk,
            }
        )
    return in_maps


def _gather_out(full):
    """full: [8*4, HF, TQ] concat of per-core out_q along axis 0."""
    full = np.asarray(full).reshape(8, 4, HF, TQ)
    out = np.empty((B, S, D), np.float32)
    for c in range(8):
        b, r = divmod(c, 4)
        for ic in range(4):
            out[b, ic * TQ:(ic + 1) * TQ, r * HF:(r + 1) * HF] = (
                full[c, ic].astype(np.float32).T
            )
    return out


def _run(inputs, trace=False):
    if "nc" not in _CACHE:
        _CACHE["nc"] = _build_graph()
    nc = _CACHE["nc"]
    in_maps = _make_in_maps(inputs)
    res = run_bass_kernel_spmd(nc, in_maps, core_ids=list(range(8)), trace=trace)

    out = np.empty((B, S, D), np.float32)
    for c in range(8):
        b, r = divmod(c, 4)
        oq = np.asarray(res.results[c]["out_q"]).astype(np.float32)
        for ic in range(4):
            out[b, ic * TQ:(ic + 1) * TQ, r * HF:(r + 1) * HF] = oq[ic].T
    return out, res


def kernel(**inputs):
    out, _ = _run(inputs, trace=False)
    return out
